# revision 2
# baseline (speedup 1.0000x reference)
"""Trainium2 Bass kernel for nn_DecoderLayer (dense transformer decoder layer).

Strategy (8 NeuronCores, full inputs in / full output out):
  - core c handles batch b = c//4 and query-quarter r = c%4 (rows [r*S/4, (r+1)*S/4)).
  - Activations are kept TRANSPOSED on-chip (x^T [D, n]) so every projection
    matmul runs with the contraction dim on partitions and fp32r (fast fp32)
    streaming at full rate with N=512 moving columns.
  - Attention per head: S^T[k, q] = K_h^T-slices.T @ Q_h^T (scores transposed),
    exp on the ACT engine (1/8 scale fused), causal/arbitrary q-k mask applied
    as data-driven multiplicative [128, W] tiles (uniform SPMD program, per-core
    mask DATA), softmax denominators obtained free by augmenting V with a ones
    column, normalization deferred to a per-head [64, W] multiply.
  - K/V are computed for the full batch (cheap, redundant across the 4 cores of
    a batch group); Q/out-proj/LayerNorm/FFN are sharded by query rows.
  - The single collective: AllGather of x1 (post-LN1) within each 4-core batch
    group, needed because cross-attention K2/V2 are projections of full x1.
  - LayerNorm runs in transposed layout: cross-partition sums via ones-matmul
    on the PE, stats broadcast back to [128, W] via ones-matmul.
"""

import sys

if "/opt/trn_rl_repo" not in sys.path:
    sys.path.insert(0, "/opt/trn_rl_repo")

import numpy as np

P = 128
HD = 64
HD1 = HD + 1
EPS = 1e-5


class Cfg:
    def __init__(self, B=2, S=2048, D=1024, H=16, DFF=4096, use_collective=True,
                 fake_gather=False):
        self.B, self.S, self.D, self.H, self.DFF = B, S, D, H, DFF
        self.fake_gather = fake_gather
        self.W = S // 4            # local query rows per core
        self.DT = D // P           # feature-dim tiles
        self.NT = S // P           # sequence tiles (keys)
        self.FT = DFF // P         # ffn hidden tiles
        self.HP = P // HD          # heads per partition-tile (2)
        self.NCH = max(1, S // 512)   # n-chunks for K-orientation matmuls
        self.NCW = S // self.NCH      # n-chunk width (<=512)
        self.VCW = min(512, D)        # v-dout chunk width
        self.VCN = D // self.VCW
        self.KTG = min(4, self.NT)    # k-tiles per exp group
        self.NG = self.NT // self.KTG
        self.use_collective = use_collective
        assert D == H * HD
        assert self.W % P == 0 and D % P == 0 and DFF % P == 0 and S % P == 0


class Flags:
    def __init__(self):
        self.qkb1 = self.vb1 = self.ob1 = False
        self.qkb2 = self.vb2 = self.ob2 = False
        self.fb1 = self.fb2 = False
        self.g1 = self.b1 = self.g2 = self.b2 = self.g3 = self.b3 = False
        self.m1 = True      # trg mask multiplicative tiles
        self.kb2 = False    # enc mask additive per-k bias


def _build(nc, tc, cfg, fl):
    import concourse.bass as bass
    import concourse.mybir as mybir
    import concourse.tile as tile  # noqa: F401
    from contextlib import ExitStack

    AF = mybir.ActivationFunctionType
    f32 = mybir.dt.float32
    f32r = mybir.dt.float32r

    def r32(ap):
        return ap.bitcast(f32r)

    B, S, D, H, DFF = cfg.B, cfg.S, cfg.D, cfg.H, cfg.DFF
    W, DT, NT, FT, HP = cfg.W, cfg.DT, cfg.NT, cfg.FT, cfg.HP
    NCH, NCW, VCW, VCN = cfg.NCH, cfg.NCW, cfg.VCW, cfg.VCN
    KTG, NG = cfg.KTG, cfg.NG
    HPC = VCW // HD  # heads per v-chunk

    # ---------------- DRAM parameters ----------------
    def din(name, shape):
        return nc.dram_tensor(name, shape, f32, kind="ExternalInput").ap()

    xT = din("xT", [D, S])
    xTl = din("xTl", [D, W])
    qkvwT1 = din("qkvwT1", [D, 3 * D])
    qkvwT2 = din("qkvwT2", [D, 3 * D])
    owT1 = din("owT1", [D, D])
    owT2 = din("owT2", [D, D])
    w1T = din("w1T", [D, DFF])
    w2T = din("w2T", [DFF, D])
    m1 = din("m1", [NT, P, W]) if fl.m1 else None
    kb2 = din("kb2", [NT, P, 1]) if fl.kb2 else None
    qkvb1 = din("qkvb1", [3 * D]) if fl.qkb1 else None
    qkvb2 = din("qkvb2", [3 * D]) if fl.qkb2 else None
    vb1 = din("vb1", [P, D]) if fl.vb1 else None
    vb2 = din("vb2", [P, D]) if fl.vb2 else None
    ob1 = din("ob1", [D]) if fl.ob1 else None
    ob2 = din("ob2", [D]) if fl.ob2 else None
    fb1d = din("fb1", [DFF]) if fl.fb1 else None
    fb2d = din("fb2", [D]) if fl.fb2 else None
    lnp = {}
    for nm, use in [("g1", fl.g1), ("b1", fl.b1), ("g2", fl.g2),
                    ("b2", fl.b2), ("g3", fl.g3), ("b3", fl.b3)]:
        lnp[nm] = din(nm, [D]) if use else None
    out = nc.dram_tensor("out", [D, W], f32, kind="ExternalOutput").ap()

    es = ExitStack()
    with es:
        dramp = es.enter_context(tc.tile_pool(name="dram", bufs=1, space="DRAM"))
        kT1s = dramp.tile([D, S], f32)
        v1s = dramp.tile([H, P, NT, HD1], f32)
        kT2s = dramp.tile([D, S], f32)
        v2s = dramp.tile([H, P, NT, HD1], f32)
        if cfg.use_collective:
            xb = dramp.tile([D, W], f32)
            agx = dramp.tile([4 * D, W], f32)

        const = es.enter_context(tc.tile_pool(name="const", bufs=1))
        ones_p1 = const.tile([P, 1], f32)
        nc.vector.memset(ones_p1[:, :], 1.0)
        ones_1p = const.tile([1, P], f32)
        nc.vector.memset(ones_1p[0:1, :], 1.0)
        eps_t = const.tile([1, 1], f32)
        nc.vector.memset(eps_t[0:1, :], EPS)

        def ldvec(dram_vec, n_tiles, name):
            """[D]-style vector -> [P, n_tiles] sbuf tile (per-partition slices)."""
            t = const.tile([P, n_tiles], f32, tag=name)
            nc.sync.dma_start(
                out=t[:, :],
                in_=dram_vec.rearrange("(t p) -> p t", p=P),
            )
            return t

        qkb1sb = ldvec(qkvb1[0 : 2 * D], 2 * DT, "qkb1") if fl.qkb1 else None
        qkb2sb = ldvec(qkvb2[0 : 2 * D], 2 * DT, "qkb2") if fl.qkb2 else None
        ob1sb = ldvec(ob1, DT, "ob1") if fl.ob1 else None
        ob2sb = ldvec(ob2, DT, "ob2") if fl.ob2 else None
        fb1sb = ldvec(fb1d, FT, "fb1") if fl.fb1 else None
        fb2sb = ldvec(fb2d, DT, "fb2") if fl.fb2 else None
        lns = {k: (ldvec(v, DT, "ln" + k) if v is not None else None)
               for k, v in lnp.items()}
        vb1sb = None
        if fl.vb1:
            vb1sb = const.tile([P, D], f32, tag="vb1")
            nc.sync.dma_start(out=vb1sb[:, :], in_=vb1[:, :])
        vb2sb = None
        if fl.vb2:
            vb2sb = const.tile([P, D], f32, tag="vb2")
            nc.sync.dma_start(out=vb2sb[:, :], in_=vb2[:, :])
        kb2sb = None
        if fl.kb2:
            kb2sb = const.tile([P, NT], f32, tag="kb2")
            nc.sync.dma_start(out=kb2sb[:, :], in_=kb2.rearrange("n p o -> p (n o)"))

        # qT/aoT/x1T are assigned later (mid pool); closures below late-bind.
        qT = aoT = x1T = None

        # =========== QKV projection phase ===========
        def qkv_phase(xk_src, xv_src, wT, kTs, vs, qkb, vbsb, x_is_sbuf):
            """xk_src(nch) -> DRAM AP [P, (DT*NCW)] (x^T n-chunk, flattened);
            xv_src(nt) -> DRAM AP [P, (DT*P)] (x^T n-tile, flattened).
            Writes K^T -> kTs dram, V(+ones) -> vs dram, Q^T local -> qT sbuf."""
            with tc.tile_pool(name="qkv_w", bufs=1) as wp, \
                 tc.tile_pool(name="qkv_xs", bufs=2) as xsp, \
                 tc.tile_pool(name="qkv_st", bufs=3) as stp, \
                 tc.tile_pool(name="qkv_ps", bufs=2, space="PSUM") as psp, \
                 tc.tile_pool(name="qkv_psq", bufs=2, space="PSUM") as psq:
                # ---- K^T [D, S] ----
                wall = wp.tile([P, DT, D], f32, tag="wall")
                nc.sync.dma_start(
                    out=r32(wall[:, :, :]),
                    in_=r32(wT[:, D : 2 * D].rearrange("(t p) v -> p t v", p=P)),
                )
                for nch in range(NCH):
                    xs = xsp.tile([P, DT, NCW], f32, tag="xs")
                    for dt in range(DT):
                        xsrc = xk_src(nch, dt)
                        xdst = (xs[:, dt, :] if len(xsrc.shape) == 2 else
                                xs[:, dt, :].rearrange("p (g w) -> p g w",
                                                       g=xsrc.shape[1]))
                        nc.sync.dma_start(out=r32(xdst), in_=r32(xsrc))
                    for dk in range(DT):
                        ps = psp.tile([P, NCW], f32, tag="kps")
                        for dt in range(DT):
                            nc.tensor.matmul(
                                ps[:, :],
                                lhsT=r32(wall[:, dt, dk * P : (dk + 1) * P]),
                                rhs=r32(xs[:, dt, :]),
                                start=(dt == 0),
                                stop=(dt == DT - 1),
                            )
                        st = stp.tile([P, NCW], f32, tag="kst")
                        if qkb is not None:
                            nc.scalar.activation(
                                out=st[:, :], in_=ps[:, :], func=AF.Identity,
                                bias=qkb[:, DT + dk : DT + dk + 1], scale=1.0,
                            )
                        else:
                            nc.scalar.activation(
                                out=st[:, :], in_=ps[:, :], func=AF.Copy,
                            )
                        nc.sync.dma_start(
                            out=kTs[dk * P : (dk + 1) * P,
                                    nch * NCW : (nch + 1) * NCW],
                            in_=st[:, :],
                        )
                # ---- V natural [n, dout] + ones column ----
                wall = wp.tile([P, DT, D], f32, tag="wall")
                nc.sync.dma_start(
                    out=r32(wall[:, :, :]),
                    in_=r32(wT[:, 2 * D : 3 * D].rearrange("(t p) v -> p t v", p=P)),
                )
                for nt in range(NT):
                    xv = xsp.tile([P, DT, P], f32, tag="xv")
                    for dt in range(DT):
                        vsrc = xv_src(nt, dt)
                        nc.sync.dma_start(out=r32(xv[:, dt, :]), in_=r32(vsrc))
                    for vc in range(VCN):
                        ps = psq.tile([P, VCW], f32, tag="vps")
                        for dt in range(DT):
                            nc.tensor.matmul(
                                ps[:, :],
                                lhsT=r32(xv[:, dt, :]),
                                rhs=r32(wall[:, dt, vc * VCW : (vc + 1) * VCW]),
                                start=(dt == 0),
                                stop=(dt == DT - 1),
                            )
                        st = stp.tile([P, HPC, HD1], f32, tag="vst")
                        nc.scalar.activation(
                            out=st[:, :, 0:HD],
                            in_=ps.rearrange("p (h d) -> p h d", d=HD),
                            func=AF.Copy,
                        )
                        if vbsb is not None:
                            nc.vector.tensor_add(
                                st[:, :, 0:HD],
                                st[:, :, 0:HD],
                                vbsb[:, vc * VCW : (vc + 1) * VCW].rearrange(
                                    "p (h d) -> p h d", d=HD),
                            )
                        nc.vector.memset(st[:, :, HD:HD1], 1.0)
                        nc.sync.dma_start(
                            out=vs[vc * HPC : (vc + 1) * HPC, :, nt, :]
                            .rearrange("h p d -> p h d"),
                            in_=st[:, :, :],
                        )
                # ---- Q^T local [D, W] ----
                wall = wp.tile([P, DT, D], f32, tag="wall")
                nc.sync.dma_start(
                    out=r32(wall[:, :, :]),
                    in_=r32(wT[:, 0:D].rearrange("(t p) v -> p t v", p=P)),
                )
                for dq in range(DT):
                    ps = psq.tile([P, W], f32, tag="qps")
                    for dt in range(DT):
                        nc.tensor.matmul(
                            ps[:, :],
                            lhsT=r32(wall[:, dt, dq * P : (dq + 1) * P]),
                            rhs=r32(x_is_sbuf[:, dt, :]),
                            start=(dt == 0),
                            stop=(dt == DT - 1),
                        )
                    if qkb is not None:
                        nc.scalar.activation(
                            out=r32(qT[:, dq, :]), in_=ps[:, :], func=AF.Identity,
                            bias=qkb[:, dq : dq + 1], scale=1.0,
                        )
                    else:
                        nc.scalar.activation(
                            out=r32(qT[:, dq, :]), in_=ps[:, :], func=AF.Copy,
                        )

        # =========== attention phase ===========
        def attn_phase(kTs, vs, m1sb, use_kb2):
            with tc.tile_pool(name="at_kv", bufs=2) as kvp, \
                 tc.tile_pool(name="at_ex", bufs=2) as exp_, \
                 tc.tile_pool(name="at_dn", bufs=2) as dnp, \
                 tc.tile_pool(name="at_ps", bufs=1, space="PSUM") as psp, \
                 tc.tile_pool(name="at_po", bufs=2, space="PSUM") as pop:
                for h in range(H):
                    hh = (h % HP) * HD  # partition base shared with q_h
                    k_h = kvp.tile([P, S], f32, tag="kh")
                    nc.sync.dma_start(
                        out=r32(k_h[hh : hh + HD, :]),
                        in_=r32(kTs[h * HD : (h + 1) * HD, :]),
                    )
                    v_h = kvp.tile([P, NT, HD1], f32, tag="vh")
                    nc.sync.dma_start(
                        out=r32(v_h[:, :, :]),
                        in_=r32(vs[h, :, :, :]),
                    )
                    q_h = qT[hh : hh + HD, h // HP, :]
                    po = pop.tile([P, W], f32, tag="po")
                    for g in range(NG):
                        ps = psp.tile([P, KTG, W], f32, tag="sc")
                        for o in range(KTG):
                            kt = g * KTG + o
                            nc.tensor.matmul(
                                ps[:, o, :],
                                lhsT=r32(k_h[hh : hh + HD, kt * P : (kt + 1) * P]),
                                rhs=r32(q_h),
                                start=True,
                                stop=True,
                            )
                        ex = exp_.tile([P, KTG, W], f32, tag="ex")
                        if use_kb2:
                            for o in range(KTG):
                                kt = g * KTG + o
                                nc.scalar.activation(
                                    out=r32(ex[:, o, :]), in_=ps[:, o, :], func=AF.Exp,
                                    bias=kb2sb[:, kt : kt + 1], scale=1.0 / np.sqrt(HD),
                                )
                        else:
                            nc.scalar.activation(
                                out=r32(ex[:, :, :]), in_=ps[:, :, :], func=AF.Exp,
                                scale=1.0 / np.sqrt(HD),
                            )
                        if m1sb is not None:
                            nc.vector.tensor_mul(
                                r32(ex[:, :, :]), ex[:, :, :],
                                m1sb[:, g * KTG : (g + 1) * KTG, :],
                            )
                        for o in range(KTG):
                            kt = g * KTG + o
                            nc.tensor.matmul(
                                po[0:HD1, :],
                                lhsT=r32(v_h[:, kt, :]),
                                rhs=r32(ex[:, o, :]),
                                start=(g == 0 and o == 0),
                                stop=(g == NG - 1 and o == KTG - 1),
                            )
                    dinv = dnp.tile([1, W], f32, tag="dinv")
                    nc.vector.reciprocal(dinv[0:1, :], po[HD:HD1, :])
                    dinvb = dnp.tile([HD, W], f32, tag="dinvb")
                    nc.gpsimd.partition_broadcast(
                        dinvb[0:HD, :], dinv[0:1, :], channels=HD
                    )
                    nc.vector.tensor_mul(
                        r32(aoT[hh : hh + HD, h // HP, :]),
                        po[0:HD, :],
                        dinvb[0:HD, :],
                    )

        # =========== layernorm (transposed layout) ===========
        def ln_t(pre, out_t, g_sb, b_sb, lpp, lp, round_out=True):
            ro = r32 if round_out else (lambda ap: ap)
            acc = lp.tile([P, W], f32, tag="lnacc")
            nc.vector.tensor_add(acc[:, :], pre[:, 0, :], pre[:, 1, :])
            for d in range(2, DT):
                nc.vector.tensor_add(acc[:, :], acc[:, :], pre[:, d, :])
            sqa = lp.tile([P, W], f32, tag="lnsqa")
            nc.scalar.square(sqa[:, :], pre[:, 0, :])
            for d in range(1, DT):
                sqt = lp.tile([P, W], f32, tag="lnsqt")
                nc.scalar.square(sqt[:, :], pre[:, d, :])
                nc.vector.tensor_add(sqa[:, :], sqa[:, :], sqt[:, :])
            sums = lpp.tile([1, W], f32, tag="lnsums")
            nc.tensor.matmul(sums[0:1, :], lhsT=ones_p1[:, :],
                             rhs=acc[:, :], start=True, stop=True)
            sqs = lpp.tile([1, W], f32, tag="lnsqs")
            nc.tensor.matmul(sqs[0:1, :], lhsT=ones_p1[:, :],
                             rhs=sqa[:, :], start=True, stop=True)
            mu = lp.tile([1, W], f32, tag="lnmu")
            nc.vector.tensor_scalar_mul(mu[0:1, :], sums[0:1, :], 1.0 / D)
            ex2 = lp.tile([1, W], f32, tag="lnex2")
            nc.vector.tensor_scalar_mul(ex2[0:1, :], sqs[0:1, :], 1.0 / D)
            mu2 = lp.tile([1, W], f32, tag="lnmu2")
            nc.scalar.square(mu2[0:1, :], mu[0:1, :])
            var = lp.tile([1, W], f32, tag="lnvar")
            nc.vector.tensor_sub(var[0:1, :], ex2[0:1, :], mu2[0:1, :])
            sd = lp.tile([1, W], f32, tag="lnsd")
            nc.scalar.activation(out=sd[0:1, :], in_=var[0:1, :], func=AF.Sqrt,
                                 bias=eps_t[0:1, :], scale=1.0)
            rstd = lp.tile([1, W], f32, tag="lnrstd")
            nc.vector.reciprocal(rstd[0:1, :], sd[0:1, :])
            mub = lpp.tile([P, W], f32, tag="lnmub")
            nc.tensor.matmul(mub[:, :], lhsT=ones_1p[0:1, :],
                             rhs=mu[0:1, :], start=True, stop=True)
            rstdb = lpp.tile([P, W], f32, tag="lnrstdb")
            nc.tensor.matmul(rstdb[:, :], lhsT=ones_1p[0:1, :],
                             rhs=rstd[0:1, :], start=True, stop=True)
            for d in range(DT):
                t1 = lp.tile([P, W], f32, tag="lnt1")
                nc.vector.tensor_sub(t1[:, :], pre[:, d, :], mub[:, :])
                nc.vector.tensor_mul(ro(out_t[:, d, :]), t1[:, :], rstdb[:, :])
                if g_sb is not None:
                    nc.vector.tensor_scalar_mul(
                        ro(out_t[:, d, :]), out_t[:, d, :], g_sb[:, d : d + 1])
                if b_sb is not None:
                    nc.vector.tensor_scalar_add(
                        ro(out_t[:, d, :]), out_t[:, d, :], b_sb[:, d : d + 1])

        # =========== out-projection + residual + LN ===========
        def proj_resid_ln(owT, obsb, residT, g_sb, b_sb, out_t):
            with tc.tile_pool(name="pr_w", bufs=2) as wp, \
                 tc.tile_pool(name="pr_t", bufs=2) as lp, \
                 tc.tile_pool(name="pr_pre", bufs=1) as prep, \
                 tc.tile_pool(name="pr_ps", bufs=2, space="PSUM") as psp, \
                 tc.tile_pool(name="pr_lnps", bufs=1, space="PSUM") as lpp:
                pre = prep.tile([P, DT, W], f32, tag="pre")
                G4 = min(4, DT)
                for dg in range(DT // G4):
                    wsl = wp.tile([P, DT, G4 * P], f32, tag="prw")
                    nc.sync.dma_start(
                        out=r32(wsl[:, :, :]),
                        in_=r32(owT[:, dg * G4 * P : (dg + 1) * G4 * P]
                                .rearrange("(t p) v -> p t v", p=P)),
                    )
                    for j in range(G4):
                        d = dg * G4 + j
                        _proj_one(d, wsl, j, obsb, residT, pre, psp, lp)
                ln_t(pre, out_t, g_sb, b_sb, lpp, lp)

        def _proj_one(d, wsl, j, obsb, residT, pre, psp, lp):
            ps = psp.tile([P, W], f32, tag="prps")
            for dt in range(DT):
                nc.tensor.matmul(
                    ps[:, :], lhsT=r32(wsl[:, dt, j * P : (j + 1) * P]),
                    rhs=r32(aoT[:, dt, :]),
                    start=(dt == 0), stop=(dt == DT - 1),
                )
            if obsb is not None:
                tmp = lp.tile([P, W], f32, tag="prtmp")
                nc.scalar.activation(out=tmp[:, :], in_=ps[:, :],
                                     func=AF.Identity,
                                     bias=obsb[:, d : d + 1], scale=1.0)
                nc.vector.tensor_add(pre[:, d, :], tmp[:, :],
                                     residT[:, d, :])
            else:
                nc.vector.tensor_add(pre[:, d, :], ps[:, :],
                                     residT[:, d, :])

        # ================= pipeline =================
        x2p = es.enter_context(tc.tile_pool(name="x2p", bufs=1))
        x2T = x2p.tile([P, DT, W], f32)

        xTr = xT.rearrange("(t p) s -> p t s", p=P)

        def xk1(nch, dt):
            return xTr[:, dt, nch * NCW : (nch + 1) * NCW]

        def xv1(nt, dt):
            return xTr[:, dt, nt * P : (nt + 1) * P]

        with tc.tile_pool(name="mid", bufs=1) as midp:
            qT = midp.tile([P, DT, W], f32)      # Q^T local (reused block2)
            aoT = midp.tile([P, DT, W], f32)     # attention out^T (reused)
            x1T = midp.tile([P, DT, W], f32)     # x1 local

            with tc.tile_pool(name="xtl", bufs=1) as xtlp:
                xTlt = xtlp.tile([P, DT, W], f32)
                nc.sync.dma_start(out=r32(xTlt[:, :, :]),
                                  in_=r32(xTl.rearrange("(t p) s -> p t s", p=P)))

                qkv_phase(xk1, xv1, qkvwT1, kT1s, v1s, qkb1sb, vb1sb, xTlt)

                if fl.m1:
                    with tc.tile_pool(name="m1p", bufs=1) as m1p:
                        m1sb = m1p.tile([P, NT, W], f32)
                        nc.sync.dma_start(out=m1sb[:, :, :],
                                          in_=m1.rearrange("n p w -> p n w"))
                        attn_phase(kT1s, v1s, m1sb, False)
                else:
                    attn_phase(kT1s, v1s, None, False)

                proj_resid_ln(owT1, ob1sb, xTlt, lns["g1"], lns["b1"], x1T)

            # ---- x1 all-gather within batch group ----
            assert cfg.use_collective
            nc.sync.dma_start(
                out=xb[:, :].rearrange("(t p) w -> p t w", p=P),
                in_=x1T[:, :, :],
            )
            if cfg.fake_gather:
                # timing-only stand-in for the collective (single-core sims)
                for g in range(4):
                    nc.sync.dma_start(out=agx[g * D : (g + 1) * D, :],
                                      in_=xb[:, :])
            else:
                nc.gpsimd.collective_compute(
                    "AllGather",
                    bass.mybir.AluOpType.bypass,
                    replica_groups=[[0, 1, 2, 3], [4, 5, 6, 7]],
                    ins=[xb[:, :]],
                    outs=[agx[:, :]],
                )
            # x1_full^T is read straight out of the gathered buffer:
            # agx rows = (g t p), columns = w; token n = g*W + w.
            agr = agx[:, :].rearrange("(g t p) w -> g p t w", g=4, p=P)

            def xk2(nch, dt):
                gs, gn = nch * NCW // W, max(1, NCW // W)
                return agr[gs : gs + gn, :, dt, :].rearrange("g p w -> p g w")

            def xv2(nt, dt):
                g, wt = (nt * P) // W, (nt * P) % W
                return agr[g, :, dt, wt : wt + P]

            qkv_phase(xk2, xv2, qkvwT2, kT2s, v2s, qkb2sb, vb2sb, x1T)

            attn_phase(kT2s, v2s, None, fl.kb2)

            proj_resid_ln(owT2, ob2sb, x1T, lns["g2"], lns["b2"], x2T)

        # ================= FFN =================
        def _ffn_resid(d, ps4, j, fb2sb, x2T, pre, lp):
            if fb2sb is not None:
                tmp = lp.tile([P, W], f32, tag="f2tmp")
                nc.scalar.activation(out=tmp[:, :], in_=ps4[j][:, :],
                                     func=AF.Identity,
                                     bias=fb2sb[:, d : d + 1], scale=1.0)
                nc.vector.tensor_add(pre[:, d, :], tmp[:, :], x2T[:, d, :])
            else:
                nc.vector.tensor_add(pre[:, d, :], ps4[j][:, :], x2T[:, d, :])

        with tc.tile_pool(name="ffh", bufs=1) as fhp, \
             tc.tile_pool(name="ffw", bufs=2) as wp, \
             tc.tile_pool(name="fft", bufs=1) as lp, \
             tc.tile_pool(name="ffpre", bufs=1) as prep:
            hT = fhp.tile([P, FT, W], f32)
            G4 = min(4, DT)
            with tc.tile_pool(name="ffps1", bufs=2, space="PSUM") as psp:
                for fg in range(FT // G4):
                    wsl = wp.tile([P, DT, G4 * P], f32, tag="f1w")
                    nc.sync.dma_start(
                        out=r32(wsl[:, :, :]),
                        in_=r32(w1T[:, fg * G4 * P : (fg + 1) * G4 * P]
                                .rearrange("(t p) v -> p t v", p=P)),
                    )
                    for j in range(G4):
                        f = fg * G4 + j
                        ps = psp.tile([P, W], f32, tag="f1ps")
                        for dt in range(DT):
                            nc.tensor.matmul(
                                ps[:, :], lhsT=r32(wsl[:, dt, j * P : (j + 1) * P]),
                                rhs=r32(x2T[:, dt, :]),
                                start=(dt == 0), stop=(dt == DT - 1),
                            )
                        if fb1sb is not None:
                            nc.scalar.activation(out=r32(hT[:, f, :]), in_=ps[:, :],
                                                 func=AF.Relu,
                                                 bias=fb1sb[:, f : f + 1], scale=1.0)
                        else:
                            nc.scalar.activation(out=r32(hT[:, f, :]), in_=ps[:, :],
                                                 func=AF.Relu)
            pre = prep.tile([P, DT, W], f32, tag="ffpre")
            with tc.tile_pool(name="ffps2", bufs=1, space="PSUM") as psq, \
                 tc.tile_pool(name="fflnps", bufs=1, space="PSUM") as lpp:
                for dg in range(DT // G4):
                    ps4 = []
                    for j in range(G4):
                        ps4j = psq.tile([P, W], f32, tag="f2ps%d" % j)
                        ps4.append(ps4j)
                    for ft in range(FT):
                        wsl = wp.tile([P, G4 * P], f32, tag="f2w")
                        nc.sync.dma_start(
                            out=r32(wsl[:, :]),
                            in_=r32(w2T[ft * P : (ft + 1) * P,
                                        dg * G4 * P : (dg + 1) * G4 * P]),
                        )
                        for j in range(G4):
                            nc.tensor.matmul(
                                ps4[j][:, :],
                                lhsT=r32(wsl[:, j * P : (j + 1) * P]),
                                rhs=r32(hT[:, ft, :]),
                                start=(ft == 0), stop=(ft == FT - 1),
                            )
                    for j in range(G4):
                        d = dg * G4 + j
                        _ffn_resid(d, ps4, j, fb2sb, x2T, pre, lp)
                ln_t(pre, pre, lns["g3"], lns["b3"], lpp, lp, round_out=False)
                for d in range(DT):
                    nc.sync.dma_start(out=out[d * P : (d + 1) * P, :],
                                      in_=pre[:, d, :])


def make_program(cfg, fl):
    from concourse import bacc
    import concourse.tile as tile

    nc = bacc.Bacc("TRN2", target_bir_lowering=False, debug=False,
                   num_devices=8)
    with tile.TileContext(nc) as tc:
        _build(nc, tc, cfg, fl)
    nc.compile()
    return nc


def prep_inputs(inputs, cfg):
    """Host-side data prep. Returns (in_maps, fl)."""
    B, S, D, H, DFF, W, NT = (cfg.B, cfg.S, cfg.D, cfg.H, cfg.DFF,
                              cfg.W, cfg.NT)
    f = np.float32
    x = np.asarray(inputs["x"], f)
    enc = np.asarray(inputs["enc_out"])
    trg = np.asarray(inputs["trg_mask"])
    fl = Flags()
    fl.qkb1 = bool(np.any(inputs["qkv_b1"]))
    fl.qkb2 = bool(np.any(inputs["qkv_b2"]))
    fl.vb1 = bool(np.any(np.asarray(inputs["qkv_b1"])[2 * D :]))
    fl.vb2 = bool(np.any(np.asarray(inputs["qkv_b2"])[2 * D :]))
    fl.ob1 = bool(np.any(inputs["out_b1"]))
    fl.ob2 = bool(np.any(inputs["out_b2"]))
    fl.fb1 = bool(np.any(inputs["ff_b1"]))
    fl.fb2 = bool(np.any(inputs["ff_b2"]))
    fl.g1 = not bool(np.all(np.asarray(inputs["ln1_g"]) == 1))
    fl.b1 = bool(np.any(inputs["ln1_b"]))
    fl.g2 = not bool(np.all(np.asarray(inputs["ln2_g"]) == 1))
    fl.b2 = bool(np.any(inputs["ln2_b"]))
    fl.g3 = not bool(np.all(np.asarray(inputs["ln3_g"]) == 1))
    fl.b3 = bool(np.any(inputs["ln3_b"]))
    fl.m1 = not bool(np.all(trg != 0))
    fl.kb2 = bool(np.any(enc == 0))

    shared = {
        "qkvwT1": np.ascontiguousarray(np.asarray(inputs["qkv_w1"], f).T),
        "qkvwT2": np.ascontiguousarray(np.asarray(inputs["qkv_w2"], f).T),
        "owT1": np.ascontiguousarray(np.asarray(inputs["out_w1"], f).T),
        "owT2": np.ascontiguousarray(np.asarray(inputs["out_w2"], f).T),
        "w1T": np.ascontiguousarray(np.asarray(inputs["ff_w1"], f).T),
        "w2T": np.ascontiguousarray(np.asarray(inputs["ff_w2"], f).T),
    }
    if fl.qkb1:
        shared["qkvb1"] = np.asarray(inputs["qkv_b1"], f)
    if fl.qkb2:
        shared["qkvb2"] = np.asarray(inputs["qkv_b2"], f)
    if fl.vb1:
        shared["vb1"] = np.broadcast_to(
            np.asarray(inputs["qkv_b1"], f)[2 * D :], (P, D)).copy()
    if fl.vb2:
        shared["vb2"] = np.broadcast_to(
            np.asarray(inputs["qkv_b2"], f)[2 * D :], (P, D)).copy()
    if fl.ob1:
        shared["ob1"] = np.asarray(inputs["out_b1"], f)
    if fl.ob2:
        shared["ob2"] = np.asarray(inputs["out_b2"], f)
    if fl.fb1:
        shared["fb1"] = np.asarray(inputs["ff_b1"], f)
    if fl.fb2:
        shared["fb2"] = np.asarray(inputs["ff_b2"], f)
    for nm, key, use in [("g1", "ln1_g", fl.g1), ("b1", "ln1_b", fl.b1),
                         ("g2", "ln2_g", fl.g2), ("b2", "ln2_b", fl.b2),
                         ("g3", "ln3_g", fl.g3), ("b3", "ln3_b", fl.b3)]:
        if use:
            shared[nm] = np.asarray(inputs[key], f)

    xTb = [np.ascontiguousarray(x[b].T) for b in range(B)]
    in_maps = []
    for c in range(8):
        b, r = c // 4, c % 4
        m = dict(shared)
        m["xT"] = xTb[b]
        m["xTl"] = np.ascontiguousarray(xTb[b][:, r * W : (r + 1) * W])
        if fl.m1:
            # m1[kt, i, j] = trg[0or b, 0, r*W + j, kt*P + i]  (0/1 float)
            tb = trg[b] if trg.shape[0] == B else trg[0]
            blk = tb[0, r * W : (r + 1) * W, :]  # [W, S] (q, k)
            m["m1"] = np.ascontiguousarray(
                (blk.T != 0).astype(f).reshape(NT, P, W))
        if fl.kb2:
            eb = enc[b, 0, 0, :]  # [S]
            m["kb2"] = np.where(eb != 0, f(0.0), f(-1e20)).astype(f).reshape(
                NT, P, 1)
        in_maps.append(m)
    return in_maps, fl


def kernel_with_results(_run_kwargs=None, **inputs):
    from concourse.bass_utils import run_bass_kernel_spmd

    cfg = Cfg()
    x = np.asarray(inputs["x"])
    assert x.shape == (cfg.B, cfg.S, cfg.D), x.shape
    in_maps, fl = prep_inputs(inputs, cfg)
    nc = make_program(cfg, fl)
    res = run_bass_kernel_spmd(nc, in_maps, list(range(8)),
                               **(_run_kwargs or {}))
    y = np.empty((cfg.B, cfg.S, cfg.D), np.float32)
    for c in range(8):
        b, r = c // 4, c % 4
        y[b, r * cfg.W : (r + 1) * cfg.W, :] = res.results[c]["out"].T
    return y, res


def kernel(**inputs):
    return kernel_with_results(**inputs)[0]



# revision 15
# speedup vs baseline: 1.7085x; 1.7085x over previous
"""Trainium2 Bass kernel for nn_DecoderLayer (dense transformer decoder layer).

Strategy (8 NeuronCores, full inputs in / full output out):
  - core c handles batch b = c//4 and query-quarter r = c%4 (rows [r*S/4, (r+1)*S/4)).
  - bf16 on every matmul path (weights, activations, K/V/Q, softmax probs):
    full PE rate, FWL weight loads, half DMA/SBUF footprint. fp32 only for
    PSUM accumulation and LayerNorm statistics.
  - K^T and V(+ones col) live entirely in SBUF (no DRAM round trip). One pass
    over x chunks produces K and V; then Q; then attention per head streams
    3-bank score groups (double buffered) through ACT exp (bf16 out) into the
    ones-augmented AV accumulation; denominators via reciprocal_approx_fast +
    GpSimd partition broadcast.
  - The single collective: bf16 AllGather of x1 (post-LN1) within each 4-core
    batch group; the L2 weight loads overlap the gather wait.
  - LayerNorm in transposed layout: cross-partition sums via f32r ones-matmuls.
"""

import sys

if "/opt/trn_rl_repo" not in sys.path:
    sys.path.insert(0, "/opt/trn_rl_repo")

import numpy as np

P = 128
HD = 64
HD1 = HD + 1
EPS = 1e-5


class Cfg:
    def __init__(self, B=2, S=2048, D=1024, H=16, DFF=4096, use_collective=True,
                 fake_gather=False, debug=False):
        self.B, self.S, self.D, self.H, self.DFF = B, S, D, H, DFF
        self.fake_gather = fake_gather
        self.debug = debug
        self.W = S // 4            # local query rows per core
        self.DT = D // P           # feature-dim tiles
        self.NT = S // P           # sequence tiles (keys)
        self.FT = DFF // P         # ffn hidden tiles
        self.HP = P // HD          # heads per partition-tile (2)
        self.NCH = max(1, S // 512)   # token chunks for K/V production
        self.NCW = S // self.NCH      # chunk width (<=512)
        self.VCW = min(512, D)        # v-dout chunk width
        self.VCN = D // self.VCW
        self.HPC = self.VCW // HD     # heads per v-chunk
        self.KTG = 3                  # k-tiles per exp group (3 banks x2 bufs)
        self.FGW = min(512, DFF)      # ffn1 out-chunk width
        self.NFG = DFF // self.FGW
        self.DGW = min(512, D)        # ffn2 out-chunk width
        self.NDG = D // self.DGW
        self.FTG = min(8, self.FT)    # ffn2 weight-chunk ft tiles
        self.use_collective = use_collective
        assert D == H * HD
        assert self.W % P == 0 and D % P == 0 and DFF % P == 0 and S % P == 0


class Flags:
    def __init__(self):
        self.qkb1 = self.vb1 = self.ob1 = False
        self.qkb2 = self.vb2 = self.ob2 = False
        self.fb1 = self.fb2 = False
        self.g1 = self.b1 = self.g2 = self.b2 = self.g3 = self.b3 = False
        self.m1 = True      # trg mask multiplicative tiles
        self.kb2 = False    # enc mask additive per-k bias


def _build(nc, tc, cfg, fl):
    import concourse.bass as bass
    import concourse.mybir as mybir
    import concourse.tile as tile  # noqa: F401
    from contextlib import ExitStack

    AF = mybir.ActivationFunctionType
    f32 = mybir.dt.float32
    f32r = mybir.dt.float32r
    bf16 = mybir.dt.bfloat16

    def r32(ap):
        return ap.bitcast(f32r)

    B, S, D, H, DFF = cfg.B, cfg.S, cfg.D, cfg.H, cfg.DFF
    W, DT, NT, FT, HP = cfg.W, cfg.DT, cfg.NT, cfg.FT, cfg.HP
    NCH, NCW, VCW, VCN, HPC = cfg.NCH, cfg.NCW, cfg.VCW, cfg.VCN, cfg.HPC
    KTG, FTG = cfg.KTG, cfg.FTG
    FGW, NFG, DGW, NDG = cfg.FGW, cfg.NFG, cfg.DGW, cfg.NDG
    NSUB = NCW // P          # token subtiles per chunk

    # ---------------- DRAM parameters ----------------
    def din(name, shape, dt=f32):
        return nc.dram_tensor(name, shape, dt, kind="ExternalInput").ap()

    wq1 = din("wq1", [P, DT, D], bf16)
    wk1 = din("wk1", [P, DT, D], bf16)
    wv1 = din("wv1", [P, DT, D], bf16)
    wo1 = din("wo1", [P, DT, D], bf16)
    wq2 = din("wq2", [P, DT, D], bf16)
    wk2 = din("wk2", [P, DT, D], bf16)
    wv2 = din("wv2", [P, DT, D], bf16)
    wo2 = din("wo2", [P, DT, D], bf16)
    w1h = din("w1h", [NFG, P, DT, FGW], bf16)
    w2h = din("w2h", [NDG, P, FT, DGW], bf16)
    xc = din("xc", [NCH, P, DT, NCW], bf16)     # x^T chunks (batch b)
    xlb = din("xlb", [P, DT, W], bf16)          # local x^T (Q rhs + residual)
    m1 = din("m1", [NT, P, W], bf16) if fl.m1 else None
    kb2 = din("kb2", [NT, P, 1]) if fl.kb2 else None
    qkvb1 = din("qkvb1", [3 * D]) if fl.qkb1 else None
    qkvb2 = din("qkvb2", [3 * D]) if fl.qkb2 else None
    vb1 = din("vb1", [P, D]) if fl.vb1 else None
    vb2 = din("vb2", [P, D]) if fl.vb2 else None
    ob1 = din("ob1", [D]) if fl.ob1 else None
    ob2 = din("ob2", [D]) if fl.ob2 else None
    fb1d = din("fb1", [DFF]) if fl.fb1 else None
    fb2d = din("fb2", [D]) if fl.fb2 else None
    lnp = {}
    for nm, use in [("g1", fl.g1), ("b1", fl.b1), ("g2", fl.g2),
                    ("b2", fl.b2), ("g3", fl.g3), ("b3", fl.b3)]:
        lnp[nm] = din(nm, [D]) if use else None
    out = nc.dram_tensor("out", [D, W], f32, kind="ExternalOutput").ap()
    dbg = {}
    if cfg.debug:
        for nm, shape, dt in [
                ("d_kT1", [P, DT, S], bf16), ("d_vsb1", [P, NT, H, HD1], bf16),
                ("d_qT1", [P, DT, W], bf16), ("d_aoT1", [P, DT, W], bf16),
                ("d_x1b", [P, DT, W], bf16),
                ("d_kT2", [P, DT, S], bf16), ("d_vsb2", [P, NT, H, HD1], bf16),
                ("d_qT2", [P, DT, W], bf16), ("d_aoT2", [P, DT, W], bf16),
                ("d_x2b", [P, DT, W], bf16), ("d_agx", [4 * D, W], bf16),
                ("d_hT", [P, FT, W], bf16)]:
            dbg[nm] = nc.dram_tensor(nm, shape, dt, kind="ExternalOutput").ap()

    es = ExitStack()
    with es:
        dramp = es.enter_context(tc.tile_pool(name="dram", bufs=1, space="DRAM"))
        if cfg.use_collective:
            xb = dramp.tile([D, W], bf16)
            agx = dramp.tile([4 * D, W], bf16)

        const = es.enter_context(tc.tile_pool(name="const", bufs=1))
        ones_p1 = const.tile([P, 1], f32)
        nc.vector.memset(ones_p1[:, :], 1.0)
        ones_1p = const.tile([1, P], f32)
        nc.vector.memset(ones_1p[0:1, :], 1.0)
        eps_t = const.tile([1, 1], f32)
        nc.vector.memset(eps_t[0:1, :], EPS)

        def ldvec(dram_vec, n_tiles, name):
            t = const.tile([P, n_tiles], f32, tag=name)
            nc.sync.dma_start(
                out=t[:, :],
                in_=dram_vec.rearrange("(t p) -> p t", p=P),
            )
            return t

        qkb1sb = ldvec(qkvb1[0 : 2 * D], 2 * DT, "qkb1") if fl.qkb1 else None
        qkb2sb = ldvec(qkvb2[0 : 2 * D], 2 * DT, "qkb2") if fl.qkb2 else None
        ob1sb = ldvec(ob1, DT, "ob1") if fl.ob1 else None
        ob2sb = ldvec(ob2, DT, "ob2") if fl.ob2 else None
        fb1sb = ldvec(fb1d, FT, "fb1") if fl.fb1 else None
        fb2sb = ldvec(fb2d, DT, "fb2") if fl.fb2 else None
        lns = {k: (ldvec(v, DT, "ln" + k) if v is not None else None)
               for k, v in lnp.items()}
        vb1sb = None
        if fl.vb1:
            vb1sb = const.tile([P, D], f32, tag="vb1")
            nc.sync.dma_start(out=vb1sb[:, :], in_=vb1[:, :])
        vb2sb = None
        if fl.vb2:
            vb2sb = const.tile([P, D], f32, tag="vb2")
            nc.sync.dma_start(out=vb2sb[:, :], in_=vb2[:, :])
        kb2sb = None
        if fl.kb2:
            kb2sb = const.tile([P, NT], f32, tag="kb2")
            nc.sync.dma_start(out=kb2sb[:, :], in_=kb2.rearrange("n p o -> p (n o)"))

        x2p = es.enter_context(tc.tile_pool(name="x2p", bufs=1))
        x2b = x2p.tile([P, DT, W], bf16)

        def dump(nm, sb_ap):
            if cfg.debug:
                nc.sync.dma_start(out=dbg[nm], in_=sb_ap)

        # =========== K/V/Q production (one pass over x chunks) ===========
        def qkv_phase(kT, vsb, qT, xk_src, q_rhs, wk_d, wv_d, wq_d, qkb, vbsb):
            nc.vector.memset(vsb[:, :, :, HD:HD1], 1.0)
            with tc.tile_pool(name="qkv_w", bufs=1) as wp, \
                 tc.tile_pool(name="qkv_xs", bufs=2) as xsp, \
                 tc.tile_pool(name="qkv_psk", bufs=2, space="PSUM") as psk, \
                 tc.tile_pool(name="qkv_psv", bufs=2, space="PSUM") as psv:
                wk = wp.tile([P, DT, D], bf16, tag="wk")
                nc.sync.dma_start(out=wk[:, :, :], in_=wk_d[:, :, :])
                wv = wp.tile([P, DT, D], bf16, tag="wv")
                nc.sync.dma_start(out=wv[:, :, :], in_=wv_d[:, :, :])
                wq = wp.tile([P, DT, D], bf16, tag="wqw")
                nc.sync.dma_start(out=wq[:, :, :], in_=wq_d[:, :, :])
                for c in range(NCH):
                    xs = xsp.tile([P, DT, NCW], bf16, tag="xs")
                    xk_src(xs, c)
                    # K^T [D, chunk]
                    for dk in range(DT):
                        ps = psk.tile([P, NCW], f32, tag="kps")
                        for dt in range(DT):
                            nc.tensor.matmul(
                                ps[:, :],
                                lhsT=wk[:, dt, dk * P : (dk + 1) * P],
                                rhs=xs[:, dt, :],
                                start=(dt == 0),
                                stop=(dt == DT - 1),
                            )
                        if qkb is not None:
                            nc.scalar.activation(
                                out=kT[:, dk, c * NCW : (c + 1) * NCW],
                                in_=ps[:, :], func=AF.Identity,
                                bias=qkb[:, DT + dk : DT + dk + 1], scale=1.0)
                        else:
                            nc.scalar.activation(
                                out=kT[:, dk, c * NCW : (c + 1) * NCW],
                                in_=ps[:, :], func=AF.Copy)
                    # V natural [token, dout] (+ones col kept intact)
                    for sub in range(NSUB):
                        nt = c * NSUB + sub
                        for vc in range(VCN):
                            ps = psv.tile([P, VCW], f32, tag="vps")
                            for dt in range(DT):
                                nc.tensor.matmul(
                                    ps[:, :],
                                    lhsT=xs[:, dt, sub * P : (sub + 1) * P],
                                    rhs=wv[:, dt, vc * VCW : (vc + 1) * VCW],
                                    start=(dt == 0),
                                    stop=(dt == DT - 1),
                                )
                            dst = vsb[:, nt, vc * HPC : (vc + 1) * HPC, 0:HD]
                            if vbsb is not None:
                                nc.vector.tensor_add(
                                    dst,
                                    ps.rearrange("p (h d) -> p h d", d=HD),
                                    vbsb[:, vc * VCW : (vc + 1) * VCW].rearrange(
                                        "p (h d) -> p h d", d=HD))
                            else:
                                nc.scalar.activation(
                                    out=dst,
                                    in_=ps.rearrange("p (h d) -> p h d", d=HD),
                                    func=AF.Copy)
                # Q^T local [D, W]
                for dq in range(DT):
                    ps = psk.tile([P, W], f32, tag="qps")
                    for dt in range(DT):
                        nc.tensor.matmul(
                            ps[:, :],
                            lhsT=wq[:, dt, dq * P : (dq + 1) * P],
                            rhs=q_rhs[:, dt, :],
                            start=(dt == 0),
                            stop=(dt == DT - 1),
                        )
                    if qkb is not None:
                        nc.scalar.activation(
                            out=qT[:, dq, :], in_=ps[:, :], func=AF.Identity,
                            bias=qkb[:, dq : dq + 1], scale=1.0)
                    else:
                        nc.scalar.activation(
                            out=qT[:, dq, :], in_=ps[:, :], func=AF.Copy)

        # =========== attention (K^T, V in SBUF) ===========
        def attn_phase(kT, vsb, qT, aoT, m1sb, use_kb2):
            scale = 1.0 / np.sqrt(HD)
            groups = []
            g0 = 0
            while g0 < NT:
                groups.append((g0, min(KTG, NT - g0)))
                g0 += KTG
            with tc.tile_pool(name="at_ex", bufs=2) as exp_, \
                 tc.tile_pool(name="at_dn", bufs=2) as dnp, \
                 tc.tile_pool(name="at_ps", bufs=2, space="PSUM") as psp, \
                 tc.tile_pool(name="at_po", bufs=2, space="PSUM") as pop:
                for h in range(H):
                    hh = (h % HP) * HD
                    q_h = qT[hh : hh + HD, h // HP, :]
                    po = pop.tile([P, W], f32, tag="po")
                    for (g0, gn) in groups:
                        ps = psp.tile([P, KTG, W], f32, tag="sc")
                        for o in range(gn):
                            kt = g0 + o
                            nc.tensor.matmul(
                                ps[:, o, :],
                                lhsT=kT[hh : hh + HD, h // HP,
                                        kt * P : (kt + 1) * P],
                                rhs=q_h,
                                start=True,
                                stop=True,
                            )
                        ex = exp_.tile([P, KTG, W], bf16, tag="ex")
                        if use_kb2:
                            for o in range(gn):
                                kt = g0 + o
                                nc.scalar.activation(
                                    out=ex[:, o, :], in_=ps[:, o, :], func=AF.Exp,
                                    bias=kb2sb[:, kt : kt + 1], scale=scale)
                        else:
                            nc.scalar.activation(
                                out=ex[:, 0:gn, :], in_=ps[:, 0:gn, :],
                                func=AF.Exp, scale=scale)
                        if m1sb is not None:
                            nc.vector.tensor_mul(
                                ex[:, 0:gn, :], ex[:, 0:gn, :],
                                m1sb[:, g0 : g0 + gn, :])
                        for o in range(gn):
                            kt = g0 + o
                            nc.tensor.matmul(
                                po[0:HD1, :],
                                lhsT=vsb[:, kt, h, :],
                                rhs=ex[:, o, :],
                                start=(kt == 0),
                                stop=(kt == NT - 1),
                            )
                    dinv = dnp.tile([1, W], f32, tag="dinv")
                    nc.vector.reciprocal(dinv[0:1, :], po[HD:HD1, :])
                    dinvb = dnp.tile([HD, W], f32, tag="dinvb")
                    nc.gpsimd.partition_broadcast(
                        dinvb[0:HD, :], dinv[0:1, :], channels=HD)
                    nc.vector.tensor_mul(
                        aoT[hh : hh + HD, h // HP, :],
                        po[0:HD, :],
                        dinvb[0:HD, :],
                    )

        # =========== layernorm (transposed layout) ===========
        def ln_t(pre, out_t, g_sb, b_sb, lpp, lp):
            acc = lp.tile([P, W], f32, tag="lnacc")
            nc.vector.tensor_add(acc[:, :], pre[:, 0, :], pre[:, 1, :])
            for d in range(2, DT):
                nc.vector.tensor_add(acc[:, :], acc[:, :], pre[:, d, :])
            sqa = lp.tile([P, W], f32, tag="lnsqa")
            nc.vector.tensor_mul(sqa[:, :], pre[:, 0, :], pre[:, 0, :])
            for d in range(1, DT):
                sqt = lp.tile([P, W], f32, tag="lnsqt")
                nc.vector.tensor_mul(sqt[:, :], pre[:, d, :], pre[:, d, :])
                nc.vector.tensor_add(sqa[:, :], sqa[:, :], sqt[:, :])
            sums = lpp.tile([1, W], f32, tag="lnsums")
            nc.tensor.matmul(sums[0:1, :], lhsT=ones_p1[:, :],
                             rhs=acc[:, :], start=True, stop=True)
            sqs = lpp.tile([1, W], f32, tag="lnsqs")
            nc.tensor.matmul(sqs[0:1, :], lhsT=ones_p1[:, :],
                             rhs=sqa[:, :], start=True, stop=True)
            mu = lp.tile([1, W], f32, tag="lnmu")
            nc.vector.tensor_scalar_mul(mu[0:1, :], sums[0:1, :], 1.0 / D)
            ex2 = lp.tile([1, W], f32, tag="lnex2")
            nc.vector.tensor_scalar_mul(ex2[0:1, :], sqs[0:1, :], 1.0 / D)
            mu2 = lp.tile([1, W], f32, tag="lnmu2")
            nc.vector.tensor_mul(mu2[0:1, :], mu[0:1, :], mu[0:1, :])
            var = lp.tile([1, W], f32, tag="lnvar")
            nc.vector.tensor_sub(var[0:1, :], ex2[0:1, :], mu2[0:1, :])
            sd = lp.tile([1, W], f32, tag="lnsd")
            nc.scalar.activation(out=sd[0:1, :], in_=var[0:1, :], func=AF.Sqrt,
                                 bias=eps_t[0:1, :], scale=1.0)
            rstd = lp.tile([1, W], f32, tag="lnrstd")
            nc.vector.reciprocal(rstd[0:1, :], sd[0:1, :])
            mub = lpp.tile([P, W], f32, tag="lnmub")
            nc.tensor.matmul(mub[:, :], lhsT=ones_1p[0:1, :],
                             rhs=mu[0:1, :], start=True, stop=True)
            rstdb = lpp.tile([P, W], f32, tag="lnrstdb")
            nc.tensor.matmul(rstdb[:, :], lhsT=ones_1p[0:1, :],
                             rhs=rstd[0:1, :], start=True, stop=True)
            for d in range(DT):
                t1 = lp.tile([P, W], f32, tag="lnt1")
                nc.vector.tensor_sub(t1[:, :], pre[:, d, :], mub[:, :])
                nc.vector.tensor_mul(out_t[:, d, :], t1[:, :], rstdb[:, :])
                if g_sb is not None:
                    nc.vector.tensor_scalar_mul(
                        out_t[:, d, :], out_t[:, d, :], g_sb[:, d : d + 1])
                if b_sb is not None:
                    nc.vector.tensor_scalar_add(
                        out_t[:, d, :], out_t[:, d, :], b_sb[:, d : d + 1])

        # =========== out-projection + residual + LN ===========
        def proj_resid_ln(aoT, wo, obsb, residT, g_sb, b_sb, out_t):
            with tc.tile_pool(name="pr_t", bufs=2) as lp, \
                 tc.tile_pool(name="pr_pre", bufs=1) as prep, \
                 tc.tile_pool(name="pr_ps", bufs=2, space="PSUM") as psp, \
                 tc.tile_pool(name="pr_lnps", bufs=1, space="PSUM") as lpp:
                pre = prep.tile([P, DT, W], f32, tag="pre")
                for d in range(DT):
                    ps = psp.tile([P, W], f32, tag="prps")
                    for dt in range(DT):
                        nc.tensor.matmul(
                            ps[:, :], lhsT=wo[:, dt, d * P : (d + 1) * P],
                            rhs=aoT[:, dt, :],
                            start=(dt == 0), stop=(dt == DT - 1),
                        )
                    if obsb is not None:
                        tmp = lp.tile([P, W], f32, tag="prtmp")
                        nc.scalar.activation(out=tmp[:, :], in_=ps[:, :],
                                             func=AF.Identity,
                                             bias=obsb[:, d : d + 1], scale=1.0)
                        nc.vector.tensor_add(pre[:, d, :], tmp[:, :],
                                             residT[:, d, :])
                    else:
                        nc.vector.tensor_add(pre[:, d, :], ps[:, :],
                                             residT[:, d, :])
                ln_t(pre, out_t, g_sb, b_sb, lpp, lp)

        # ================= pipeline =================
        with tc.tile_pool(name="kv", bufs=1) as kvp:
            kT = kvp.tile([P, DT, S], bf16)
            vsb = kvp.tile([P, NT, H, HD1], bf16)
            qT = kvp.tile([P, DT, W], bf16)
            aoT = kvp.tile([P, DT, W], bf16)
            x1b = kvp.tile([P, DT, W], bf16)

            with tc.tile_pool(name="xtl", bufs=1) as xtlp:
                xlbt = xtlp.tile([P, DT, W], bf16)
                nc.sync.dma_start(out=xlbt[:, :, :], in_=xlb[:, :, :])

                def xk1(xs, c):
                    nc.sync.dma_start(out=xs[:, :, :], in_=xc[c, :, :, :])

                qkv_phase(kT, vsb, qT, xk1, xlbt,
                          wk1, wv1, wq1, qkb1sb, vb1sb)
                dump("d_kT1", kT[:, :, :])
                dump("d_vsb1", vsb[:, :, :, :])
                dump("d_qT1", qT[:, :, :])

                with tc.tile_pool(name="wo1p", bufs=1) as wop:
                    wo1t = wop.tile([P, DT, D], bf16)
                    nc.sync.dma_start(out=wo1t[:, :, :], in_=wo1[:, :, :])
                    if fl.m1:
                        with tc.tile_pool(name="m1p", bufs=1) as m1p:
                            m1sb = m1p.tile([P, NT, W], bf16)
                            nc.sync.dma_start(
                                out=m1sb[:, :, :],
                                in_=m1.rearrange("n p w -> p n w"))
                            attn_phase(kT, vsb, qT, aoT, m1sb, False)
                    else:
                        attn_phase(kT, vsb, qT, aoT, None, False)
                    dump("d_aoT1", aoT[:, :, :])
                    proj_resid_ln(aoT, wo1t, ob1sb, xlbt,
                                  lns["g1"], lns["b1"], x1b)
                    dump("d_x1b", x1b[:, :, :])

            # ---- x1 all-gather (bf16) within batch group ----
            assert cfg.use_collective
            nc.sync.dma_start(
                out=xb[:, :].rearrange("(t p) w -> p t w", p=P),
                in_=x1b[:, :, :],
            )
            if cfg.fake_gather:
                for g in range(4):
                    nc.sync.dma_start(out=agx[g * D : (g + 1) * D, :],
                                      in_=xb[:, :])
            else:
                nc.gpsimd.collective_compute(
                    "AllGather",
                    bass.mybir.AluOpType.bypass,
                    replica_groups=[[0, 1, 2, 3], [4, 5, 6, 7]],
                    ins=[xb[:, :]],
                    outs=[agx[:, :]],
                )
            def xk2(xs, c):
                gs, gn = c * NCW // W, max(1, NCW // W)
                for g in range(gn):
                    src = agx[(gs + g) * D : (gs + g + 1) * D, :].rearrange(
                        "(t p) w -> p t w", p=P)
                    nc.sync.dma_start(
                        out=xs[:, :, g * W : (g + 1) * W], in_=src)

            if cfg.debug:
                nc.sync.dma_start(out=dbg["d_agx"], in_=agx[:, :])
            qkv_phase(kT, vsb, qT, xk2, x1b, wk2, wv2, wq2, qkb2sb, vb2sb)
            dump("d_kT2", kT[:, :, :])
            dump("d_vsb2", vsb[:, :, :, :])
            dump("d_qT2", qT[:, :, :])

            with tc.tile_pool(name="wo2p", bufs=1) as wop:
                wo2t = wop.tile([P, DT, D], bf16)
                nc.sync.dma_start(out=wo2t[:, :, :], in_=wo2[:, :, :])
                attn_phase(kT, vsb, qT, aoT, None, fl.kb2)
                dump("d_aoT2", aoT[:, :, :])
                proj_resid_ln(aoT, wo2t, ob2sb, x1b,
                              lns["g2"], lns["b2"], x2b)
                dump("d_x2b", x2b[:, :, :])

        # ================= FFN =================
        with tc.tile_pool(name="ffh", bufs=1) as fhp, \
             tc.tile_pool(name="ffw", bufs=2) as wp, \
             tc.tile_pool(name="fft", bufs=2) as lp, \
             tc.tile_pool(name="ffpre", bufs=1) as prep:
            hT = fhp.tile([P, FT, W], bf16)
            with tc.tile_pool(name="ffps1", bufs=2, space="PSUM") as psp:
                for fg in range(NFG):
                    wsl = wp.tile([P, DT, FGW], bf16, tag="f1w")
                    nc.sync.dma_start(out=wsl[:, :, :], in_=w1h[fg, :, :, :])
                    for j in range(FGW // P):
                        f = fg * (FGW // P) + j
                        ps = psp.tile([P, W], f32, tag="f1ps")
                        for dt in range(DT):
                            nc.tensor.matmul(
                                ps[:, :], lhsT=wsl[:, dt, j * P : (j + 1) * P],
                                rhs=x2b[:, dt, :],
                                start=(dt == 0), stop=(dt == DT - 1),
                            )
                        if fb1sb is not None:
                            nc.scalar.activation(out=hT[:, f, :], in_=ps[:, :],
                                                 func=AF.Relu,
                                                 bias=fb1sb[:, f : f + 1],
                                                 scale=1.0)
                        else:
                            nc.scalar.activation(out=hT[:, f, :], in_=ps[:, :],
                                                 func=AF.Relu)
            dump("d_hT", hT[:, :, :])
            pre = prep.tile([P, DT, W], f32, tag="ffpre")
            with tc.tile_pool(name="ffps2", bufs=1, space="PSUM") as psq, \
                 tc.tile_pool(name="fflnps", bufs=1, space="PSUM") as lpp:
                for dg in range(NDG):
                    nj = DGW // P
                    ps4 = []
                    for j in range(nj):
                        ps4j = psq.tile([P, W], f32, tag="f2ps%d" % j)
                        ps4.append(ps4j)
                    for ftg in range(FT // FTG):
                        wsl = wp.tile([P, FTG, DGW], bf16, tag="f2w")
                        nc.sync.dma_start(
                            out=wsl[:, :, :],
                            in_=w2h[dg, :, ftg * FTG : (ftg + 1) * FTG, :])
                        for fo in range(FTG):
                            ft = ftg * FTG + fo
                            for j in range(nj):
                                nc.tensor.matmul(
                                    ps4[j][:, :],
                                    lhsT=wsl[:, fo, j * P : (j + 1) * P],
                                    rhs=hT[:, ft, :],
                                    start=(ft == 0), stop=(ft == FT - 1),
                                )
                    for j in range(nj):
                        d = dg * nj + j
                        if fb2sb is not None:
                            tmp = lp.tile([P, W], f32, tag="f2tmp")
                            nc.scalar.activation(out=tmp[:, :], in_=ps4[j][:, :],
                                                 func=AF.Identity,
                                                 bias=fb2sb[:, d : d + 1],
                                                 scale=1.0)
                            nc.vector.tensor_add(pre[:, d, :], tmp[:, :],
                                                 x2b[:, d, :])
                        else:
                            nc.vector.tensor_add(pre[:, d, :], ps4[j][:, :],
                                                 x2b[:, d, :])
                ln_t(pre, pre, lns["g3"], lns["b3"], lpp, lp)
                for d in range(DT):
                    nc.sync.dma_start(out=out[d * P : (d + 1) * P, :],
                                      in_=pre[:, d, :])


def make_program(cfg, fl):
    from concourse import bacc
    import concourse.tile as tile

    nc = bacc.Bacc("TRN2", target_bir_lowering=False, debug=False,
                   num_devices=8)
    with tile.TileContext(nc) as tc:
        _build(nc, tc, cfg, fl)
    nc.compile()
    return nc


def prep_inputs(inputs, cfg):
    """Host-side data prep. Returns (in_maps, fl)."""
    import ml_dtypes

    B, S, D, H, DFF, W, NT, DT, FT = (cfg.B, cfg.S, cfg.D, cfg.H, cfg.DFF,
                                      cfg.W, cfg.NT, cfg.DT, cfg.FT)
    NCH, NCW = cfg.NCH, cfg.NCW
    NFG, FGW, NDG, DGW = cfg.NFG, cfg.FGW, cfg.NDG, cfg.DGW
    f = np.float32
    bf = ml_dtypes.bfloat16
    x = np.asarray(inputs["x"], f)
    enc = np.asarray(inputs["enc_out"])
    trg = np.asarray(inputs["trg_mask"])
    fl = Flags()
    fl.qkb1 = bool(np.any(inputs["qkv_b1"]))
    fl.qkb2 = bool(np.any(inputs["qkv_b2"]))
    fl.vb1 = bool(np.any(np.asarray(inputs["qkv_b1"])[2 * D :]))
    fl.vb2 = bool(np.any(np.asarray(inputs["qkv_b2"])[2 * D :]))
    fl.ob1 = bool(np.any(inputs["out_b1"]))
    fl.ob2 = bool(np.any(inputs["out_b2"]))
    fl.fb1 = bool(np.any(inputs["ff_b1"]))
    fl.fb2 = bool(np.any(inputs["ff_b2"]))
    fl.g1 = not bool(np.all(np.asarray(inputs["ln1_g"]) == 1))
    fl.b1 = bool(np.any(inputs["ln1_b"]))
    fl.g2 = not bool(np.all(np.asarray(inputs["ln2_g"]) == 1))
    fl.b2 = bool(np.any(inputs["ln2_b"]))
    fl.g3 = not bool(np.all(np.asarray(inputs["ln3_g"]) == 1))
    fl.b3 = bool(np.any(inputs["ln3_b"]))
    fl.m1 = not bool(np.all(trg != 0))
    fl.kb2 = bool(np.any(enc == 0))

    def pmaj(w):
        # [Dout, Din] -> [P, DT_in, Dout] bf16 with the contraction (Din)
        # index on partitions: pmaj(w)[p, t, v] = w[v, t*P+p]
        wT = np.ascontiguousarray(w.T)
        din = wT.shape[0]
        return np.ascontiguousarray(
            wT.reshape(din // P, P, wT.shape[1]).transpose(1, 0, 2)).astype(bf)

    qw1 = np.asarray(inputs["qkv_w1"], f)
    qw2 = np.asarray(inputs["qkv_w2"], f)
    shared = {
        "wq1": pmaj(qw1[0:D]), "wk1": pmaj(qw1[D : 2 * D]),
        "wv1": pmaj(qw1[2 * D : 3 * D]),
        "wq2": pmaj(qw2[0:D]), "wk2": pmaj(qw2[D : 2 * D]),
        "wv2": pmaj(qw2[2 * D : 3 * D]),
        "wo1": pmaj(np.asarray(inputs["out_w1"], f)),
        "wo2": pmaj(np.asarray(inputs["out_w2"], f)),
    }
    # FFN1: w1h[fg, p, dt, j] = ff_w1[fg*FGW+j, dt*P+p]
    w1 = np.asarray(inputs["ff_w1"], f)   # [DFF, D]
    shared["w1h"] = np.ascontiguousarray(
        w1.reshape(NFG, FGW, DT, P).transpose(0, 3, 2, 1)).astype(bf)
    # FFN2: w2h[dg, p, ft, j] = ff_w2.T[ft*P+p, dg*DGW+j]
    w2T = np.asarray(inputs["ff_w2"], f).T   # [DFF, D]
    shared["w2h"] = np.ascontiguousarray(
        w2T.reshape(FT, P, NDG, DGW).transpose(2, 1, 0, 3)).astype(bf)
    if fl.qkb1:
        shared["qkvb1"] = np.asarray(inputs["qkv_b1"], f)
    if fl.qkb2:
        shared["qkvb2"] = np.asarray(inputs["qkv_b2"], f)
    if fl.vb1:
        shared["vb1"] = np.broadcast_to(
            np.asarray(inputs["qkv_b1"], f)[2 * D :], (P, D)).copy()
    if fl.vb2:
        shared["vb2"] = np.broadcast_to(
            np.asarray(inputs["qkv_b2"], f)[2 * D :], (P, D)).copy()
    if fl.ob1:
        shared["ob1"] = np.asarray(inputs["out_b1"], f)
    if fl.ob2:
        shared["ob2"] = np.asarray(inputs["out_b2"], f)
    if fl.fb1:
        shared["fb1"] = np.asarray(inputs["ff_b1"], f)
    if fl.fb2:
        shared["fb2"] = np.asarray(inputs["ff_b2"], f)
    for nm, key, use in [("g1", "ln1_g", fl.g1), ("b1", "ln1_b", fl.b1),
                         ("g2", "ln2_g", fl.g2), ("b2", "ln2_b", fl.b2),
                         ("g3", "ln3_g", fl.g3), ("b3", "ln3_b", fl.b3)]:
        if use:
            shared[nm] = np.asarray(inputs[key], f)

    in_maps = []
    for c in range(8):
        b, r = c // 4, c % 4
        m = dict(shared)
        xT = x[b].T                      # [D, S]
        # xc[c][p, t, w] = xT[t*P+p, c*NCW+w]
        m["xc"] = np.ascontiguousarray(
            xT.reshape(DT, P, NCH, NCW).transpose(2, 1, 0, 3)).astype(bf)
        xl = xT[:, r * W : (r + 1) * W]  # [D, W]
        m["xlb"] = np.ascontiguousarray(
            xl.reshape(DT, P, W).transpose(1, 0, 2)).astype(bf)
        if fl.m1:
            tb = trg[b] if trg.shape[0] == B else trg[0]
            blk = tb[0, r * W : (r + 1) * W, :]  # [W, S] (q, k)
            m["m1"] = np.ascontiguousarray(
                (blk.T != 0).astype(f).reshape(NT, P, W)).astype(bf)
        if fl.kb2:
            eb = enc[b, 0, 0, :]
            m["kb2"] = np.where(eb != 0, f(0.0), f(-1e20)).astype(f).reshape(
                NT, P, 1)
        in_maps.append(m)
    return in_maps, fl


def kernel_with_results(_run_kwargs=None, **inputs):
    from concourse.bass_utils import run_bass_kernel_spmd

    cfg = Cfg()
    x = np.asarray(inputs["x"])
    assert x.shape == (cfg.B, cfg.S, cfg.D), x.shape
    in_maps, fl = prep_inputs(inputs, cfg)
    nc = make_program(cfg, fl)
    res = run_bass_kernel_spmd(nc, in_maps, list(range(8)),
                               **(_run_kwargs or {}))
    y = np.empty((cfg.B, cfg.S, cfg.D), np.float32)
    for c in range(8):
        b, r = c // 4, c % 4
        y[b, r * cfg.W : (r + 1) * cfg.W, :] = res.results[c]["out"].T
    return y, res


def kernel(**inputs):
    return kernel_with_results(**inputs)[0]
